# revision 6
# baseline (speedup 1.0000x reference)
"""DiffNet (social-diffusion recsys) forward pass as a Bass/Tile kernel on 8
Trainium2 NeuronCores.

Strategy: data-parallel over the batch (16384 rows -> 2048 rows/core), tables
replicated in every core's HBM. The HW gather primitive on this stack is an
SWDGE indirect DMA with one int32 index per partition (a standalone
contiguous [128,1] SBUF index tile) fetching one table row per partition —
128 rows/instruction. Per core:

  - rows are sorted by neighbor count (host-side permutation, undone on the
    way out), so the k-th neighbor gather is only issued for slots whose max
    count exceeds k: ~Sum_t max_len(slot t) ~= 425 instead of 16*50=800
    gather instructions.
  - neighbor sums accumulate directly in the DMA (CCE add) into per-slot
    [128,66] accumulators; padded slots carry an out-of-range index and are
    skipped by the DGE bounds check (the accumulator is pre-zeroed).
  - user/product bias tables and the global bias are folded into augmented
    66-column tables ([emb, ub+gb, 1] / [emb, 1, pb]), so the final biased
    score is a single 66-wide dot product.
  - the two diffusion layers run feature-major ([64, 2048]) via PE
    transposes, 64x64 matmuls, fused bias+ReLU on ACT, and a predicated
    copy for the empty-neighbor passthrough.

Batch row r of a core maps to (partition p, slot t) with r = t*128 + p.
"""

import os
import sys

for _p in ("/opt/trn_rl_repo",):
    if os.path.isdir(_p) and _p not in sys.path:
        sys.path.append(_p)

import numpy as np
from contextlib import ExitStack

import concourse.bass as bass
import concourse.bacc as bacc
import concourse.tile as tile
from concourse import mybir
from concourse.bass_utils import run_bass_kernel_spmd
from concourse.masks import make_identity

N_USERS = 1_000_000
N_PRODUCTS = 500_000
N_CAT = 1_000
F = 64
FA = 66                   # augmented row: emb(64) + 2 bias lanes
L = 2
B = 16384
K = 50

NCORES = 8
BC = B // NCORES          # 2048 rows per core
P = 128                   # partitions
T = BC // P               # 16 slots per partition
CH = 4 * P                # 512 transposed columns per matmul chunk
NCH = BC // CH            # 4 chunks

dt = mybir.dt


def _build_program(k_sched):
    """k_sched: tuple of T ints — number of neighbor gathers per slot."""
    nc = bacc.Bacc("TRN2", target_bir_lowering=False, debug=False)

    NG_N = sum(k_sched)
    u66 = nc.dram_tensor("u66", [N_USERS, FA], dt.float32, kind="ExternalInput").ap()
    p66 = nc.dram_tensor("p66", [N_PRODUCTS, FA], dt.float32, kind="ExternalInput").ap()
    c66 = nc.dram_tensor("c66", [N_CAT, FA], dt.float32, kind="ExternalInput").ap()
    wt = nc.dram_tensor("wt", [L, F, F], dt.float32, kind="ExternalInput").ap()
    bv = nc.dram_tensor("bv", [L, F], dt.float32, kind="ExternalInput").ap()
    # gather index columns: [3*T + NG_N, 128] — u, p, c, then neighbor cols
    idxp = nc.dram_tensor("idxp", [3 * T + NG_N, P], dt.int32, kind="ExternalInput").ap()
    invc = nc.dram_tensor("invc", [P, T], dt.float32, kind="ExternalInput").ap()
    hnot = nc.dram_tensor("hnot", [1, BC], dt.float32, kind="ExternalInput").ap()
    out_d = nc.dram_tensor("out", [T, P], dt.float32, kind="ExternalOutput").ap()

    f32 = dt.float32
    AX = mybir.AxisListType
    OP = mybir.AluOpType
    AF = mybir.ActivationFunctionType

    with tile.TileContext(nc) as tc, ExitStack() as ctx:
        sp = ctx.enter_context(tc.tile_pool(name="s", bufs=1))
        ixp = ctx.enter_context(tc.tile_pool(name="ix", bufs=48))
        ptp = ctx.enter_context(tc.tile_pool(name="ptp", bufs=2, space="PSUM"))
        pmm = ctx.enter_context(tc.tile_pool(name="pmm", bufs=2, space="PSUM"))

        ident = sp.tile([P, P], f32)
        make_identity(nc, ident[:])
        ones1 = sp.tile([1, F], f32)
        nc.vector.memset(ones1[:], 1.0)

        t_invc = sp.tile([P, T], f32)
        nc.sync.dma_start(out=t_invc[:], in_=invc[:, :])
        t_hnot = sp.tile([1, BC], f32)
        nc.sync.dma_start(out=t_hnot[:], in_=hnot[:, :])
        t_wt = sp.tile([F, L, F], f32)
        nc.sync.dma_start(out=t_wt[:], in_=wt.rearrange("l i o -> i l o"))
        t_b = sp.tile([F, L], f32)
        nc.sync.dma_start(out=t_b[:], in_=bv.rearrange("l f -> f l"))

        hw_q = [nc.sync, nc.scalar]     # alternate HWDGE queues for idx loads
        n_idx_dma = [0]

        def idx_tile(col):
            ix = ixp.tile([P, 1], dt.int32, name=f"ix{col}", tag="ix")
            eng = hw_q[n_idx_dma[0] % 2]
            n_idx_dma[0] += 1
            eng.dma_start(out=ix[:], in_=idxp[col, :].rearrange("(p o) -> p o", o=1))
            return ix

        def gather(out_tile, table, col, acc=False, bounds=None):
            ix = idx_tile(col)
            kw = {}
            if acc:
                kw["compute_op"] = OP.add
            if bounds is not None:
                kw["bounds_check"] = bounds
                kw["oob_is_err"] = False
            nc.gpsimd.indirect_dma_start(
                out=out_tile[:], out_offset=None, in_=table[:],
                in_offset=bass.IndirectOffsetOnAxis(ap=ix[:], axis=0), **kw)

        # ---- row gathers: u66 / p66(+c66) per slot --------------------
        t_u = [sp.tile([P, FA], f32, name=f"tu{t}", tag=f"tu{t}") for t in range(T)]
        t_pc = [sp.tile([P, FA], f32, name=f"tpc{t}", tag=f"tpc{t}") for t in range(T)]
        for t in range(T):
            gather(t_u[t], u66, t)
            gather(t_pc[t], p66, T + t)
            gather(t_pc[t], c66, 2 * T + t, acc=True)

        # ---- neighbor accumulate gathers (k-major interleave) ---------
        t_acc = [sp.tile([P, FA], f32, name=f"ta{t}", tag=f"ta{t}") for t in range(T)]
        for t in range(T):
            nc.vector.memset(t_acc[t][:], 0.0)
        colbase = []
        c = 3 * T
        for t in range(T):
            colbase.append(c)
            c += k_sched[t]
        kmax = max(k_sched) if k_sched else 0
        for k in range(kmax):
            for t in range(T):
                if k < k_sched[t]:
                    gather(t_acc[t], u66, colbase[t] + k, acc=True,
                           bounds=N_USERS - 1)

        # ---- has_nbr==0 mask broadcast to [64, BC] via K=1 matmul -----
        # (copy_predicated wants an integer mask -> cast on the PSUM copy)
        t_m0 = sp.tile([F, BC], dt.uint8)
        for cch in range(NCH):
            pm = pmm.tile([F, CH], f32, tag="mm")
            nc.tensor.matmul(pm[:], lhsT=ones1[:], rhs=t_hnot[:, cch * CH:(cch + 1) * CH],
                             start=True, stop=True)
            nc.vector.tensor_copy(out=t_m0[:, cch * CH:(cch + 1) * CH], in_=pm[:])

        # ---- neighbor mean + transposes into feature-major ------------
        t_nm = sp.tile([P, T, F], f32)
        t_uT = sp.tile([F, BC], f32)
        t_nmT = sp.tile([F, BC], f32)
        for t in range(T):
            nc.vector.tensor_scalar(
                out=t_nm[:, t, :], in0=t_acc[t][:, 0:F],
                scalar1=t_invc[:, t:t + 1], scalar2=None, op0=OP.mult)
            cs = slice(t * P, (t + 1) * P)
            pt = ptp.tile([F, P], f32, tag="tp")
            nc.tensor.transpose(out=pt[:], in_=t_u[t][:, 0:F], identity=ident[:])
            nc.scalar.copy(out=t_uT[:, cs], in_=pt[:])
            pt2 = ptp.tile([F, P], f32, tag="tp")
            nc.tensor.transpose(out=pt2[:], in_=t_nm[:, t, :], identity=ident[:])
            nc.scalar.copy(out=t_nmT[:, cs], in_=pt2[:])

        # ---- diffusion layers (feature-major) -------------------------
        t_xT = sp.tile([F, BC], f32)
        cur = t_uT
        nxt = sp.tile([F, BC], f32, name="t_uT2", tag="t_uT2")
        for l in range(L):
            for cch in range(NCH):
                cs = slice(cch * CH, (cch + 1) * CH)
                nc.vector.tensor_tensor(out=t_xT[:, cs], in0=cur[:, cs],
                                        in1=t_nmT[:, cs], op=OP.add)
                pm = pmm.tile([F, CH], f32, tag="mm")
                nc.tensor.matmul(pm[:], lhsT=t_wt[:, l, :], rhs=t_xT[:, cs],
                                 start=True, stop=True)
                nc.scalar.activation(out=nxt[:, cs], in_=pm[:], func=AF.Relu,
                                     bias=t_b[:, l:l + 1])
                nc.vector.copy_predicated(out=nxt[:, cs], mask=t_m0[:, cs],
                                          data=cur[:, cs])
            cur, nxt = nxt, cur

        # ---- back to row-major, 66-wide dot (biases included) ---------
        t_uf = sp.tile([P, T, FA], f32)
        for t in range(T):
            pt3 = ptp.tile([P, F], f32, tag="tb")
            nc.tensor.transpose(out=pt3[:], in_=cur[:, t * P:(t + 1) * P],
                                identity=ident[:F, :F])
            nc.scalar.copy(out=t_uf[:, t, 0:F], in_=pt3[:])
            nc.vector.tensor_copy(out=t_uf[:, t, F:FA], in_=t_u[t][:, F:FA])

        t_pcp = sp.tile([P, T, FA], f32)
        for t in range(T):
            nc.vector.tensor_copy(out=t_pcp[:, t, :], in_=t_pc[t][:])
        nc.vector.tensor_tensor(out=t_pcp[:], in0=t_pcp[:], in1=t_uf[:], op=OP.mult)
        t_int = sp.tile([P, T], f32)
        nc.vector.tensor_reduce(out=t_int[:], in_=t_pcp[:], axis=AX.X, op=OP.add)
        nc.sync.dma_start(out=out_d.rearrange("t p -> p t"), in_=t_int[:])

    nc.compile()
    return nc


_PROGRAM_CACHE = {}


def _get_program(k_sched):
    key = tuple(k_sched)
    if key not in _PROGRAM_CACHE:
        _PROGRAM_CACHE[key] = _build_program(key)
    return _PROGRAM_CACHE[key]


def kernel(user_idx, product_idx, category_idx, neighbor_idx, neighbor_lens,
           user_emb, product_emb, category_emb, user_bias_tab, product_bias_tab,
           global_bias, W, b, _run_kwargs=None, _return_res=False):
    user_idx = np.asarray(user_idx).astype(np.int32)
    product_idx = np.asarray(product_idx).astype(np.int32)
    category_idx = np.asarray(category_idx).astype(np.int32)
    neighbor_idx = np.asarray(neighbor_idx).astype(np.int32)
    neighbor_lens = np.asarray(neighbor_lens).astype(np.int64)
    user_emb = np.asarray(user_emb, dtype=np.float32)
    product_emb = np.asarray(product_emb, dtype=np.float32)
    category_emb = np.asarray(category_emb, dtype=np.float32)
    user_bias_tab = np.asarray(user_bias_tab, dtype=np.float32)
    product_bias_tab = np.asarray(product_bias_tab, dtype=np.float32)
    gb = float(np.asarray(global_bias, dtype=np.float32))
    W = np.asarray(W, dtype=np.float32)
    b = np.asarray(b, dtype=np.float32)

    # augmented tables: score = dot66(u66_final, p66+c66)
    u66_t = np.empty((N_USERS, FA), np.float32)
    u66_t[:, :F] = user_emb
    u66_t[:, F] = user_bias_tab + gb
    u66_t[:, F + 1] = 1.0
    p66_t = np.empty((N_PRODUCTS, FA), np.float32)
    p66_t[:, :F] = product_emb
    p66_t[:, F] = 1.0
    p66_t[:, F + 1] = product_bias_tab
    c66_t = np.zeros((N_CAT, FA), np.float32)
    c66_t[:, :F] = 0.3 * category_emb

    lens = np.clip(neighbor_lens, 0, K).astype(np.int64)

    # per-core sort by neighbor count; schedule shared across cores
    perms, kslots = [], np.zeros((NCORES, T), np.int64)
    for c in range(NCORES):
        lc = lens[c * BC:(c + 1) * BC]
        perm = np.argsort(lc, kind="stable")
        perms.append(perm)
        ls = lc[perm]
        kslots[c] = ls.reshape(T, P).max(axis=1)
    k_sched = tuple(int(x) for x in kslots.max(axis=0))
    nc = _get_program(k_sched)

    in_maps = []
    for c in range(NCORES):
        sl = slice(c * BC, (c + 1) * BC)
        perm = perms[c]
        ui = user_idx[sl][perm]
        pi = product_idx[sl][perm]
        ci = category_idx[sl][perm]
        ni = neighbor_idx[sl][perm]          # [BC, K]
        lc = lens[sl][perm]

        cols = [ui.reshape(T, P), pi.reshape(T, P), ci.reshape(T, P)]
        ncols = []
        ni3 = ni.reshape(T, P, K)
        lc2 = lc.reshape(T, P)
        for t in range(T):
            kk = k_sched[t]
            col = np.where(np.arange(kk)[None, :] < lc2[t][:, None],
                           ni3[t, :, :kk], N_USERS).astype(np.int32)  # [P, kk]
            ncols.append(col.T)              # [kk, P]
        idxp_np = np.concatenate([np.concatenate(cols, 0).astype(np.int32)]
                                 + ncols, axis=0)

        invc_np = (1.0 / np.maximum(lc2, 1)).astype(np.float32).T.copy()  # [P,T]
        hnot_np = (lc == 0).astype(np.float32).reshape(1, BC)

        in_maps.append({
            "u66": u66_t, "p66": p66_t, "c66": c66_t,
            "wt": np.ascontiguousarray(W.transpose(0, 2, 1)),
            "bv": np.ascontiguousarray(b),
            "idxp": np.ascontiguousarray(idxp_np),
            "invc": np.ascontiguousarray(invc_np),
            "hnot": hnot_np,
        })

    res = run_bass_kernel_spmd(nc, in_maps, list(range(NCORES)),
                               **(_run_kwargs or {}))
    out = np.empty(B, np.float32)
    for c in range(NCORES):
        o = res.results[c]["out"].reshape(-1)   # sorted order, r = t*128+p
        dst = out[c * BC:(c + 1) * BC]
        dst[perms[c]] = o
    if _return_res:
        return out, res
    return out


# revision 15
# speedup vs baseline: 1.3768x; 1.3768x over previous
"""DiffNet (social-diffusion recsys) forward pass as a Bass/Tile kernel on 8
Trainium2 NeuronCores.

Strategy: data-parallel over the batch (16384 rows -> 2048 rows/core), tables
replicated in every core's HBM. The HW gather primitive on this stack is an
SWDGE indirect DMA with one int32 index per partition (a standalone
contiguous [128,1] SBUF index tile) fetching one table row per partition —
128 rows/instruction. Per core:

  - rows are sorted by neighbor count (host-side permutation, undone on the
    way out), so the k-th neighbor gather is only issued for slots whose max
    count exceeds k: ~Sum_t max_len(slot t) ~= 425 instead of 16*50=800
    gather instructions.
  - neighbor sums accumulate directly in the DMA (CCE add) into per-slot
    [128,66] accumulators; padded slots carry an out-of-range index and are
    skipped by the DGE bounds check (the accumulator is pre-zeroed).
  - user/product bias tables and the global bias are folded into augmented
    66-column tables ([emb, ub+gb, 1] / [emb, 1, pb]), so the final biased
    score is a single 66-wide dot product.
  - the two diffusion layers run feature-major ([64, 2048]) via PE
    transposes, 64x64 matmuls, fused bias+ReLU on ACT, and a predicated
    copy for the empty-neighbor passthrough.

Batch row r of a core maps to (partition p, slot t) with r = t*128 + p.
"""

import os
import sys

for _p in ("/opt/trn_rl_repo",):
    if os.path.isdir(_p) and _p not in sys.path:
        sys.path.append(_p)

import numpy as np
from contextlib import ExitStack

import concourse.bass as bass
import concourse.bacc as bacc
import concourse.tile as tile
from concourse import mybir
from concourse.bass_utils import run_bass_kernel_spmd
from concourse.masks import make_identity

N_USERS = 1_000_000
N_PRODUCTS = 500_000
N_CAT = 1_000
F = 64
FA = 66                   # augmented row: emb(64) + 2 bias lanes
L = 2
B = 16384
K = 50

NCORES = 8
BC = B // NCORES          # 2048 rows per core
P = 128                   # partitions
T = BC // P               # 16 slots per partition
CH = 4 * P                # 512 transposed columns per matmul chunk
NCH = BC // CH            # 4 chunks

dt = mybir.dt


def _build_program(k_sched):
    """k_sched: tuple of T ints — number of neighbor gathers per slot."""
    nc = bacc.Bacc("TRN2", target_bir_lowering=False, debug=False)

    NG_N = sum(k_sched)
    u66 = nc.dram_tensor("u66", [N_USERS, FA], dt.float32, kind="ExternalInput").ap()
    p66 = nc.dram_tensor("p66", [N_PRODUCTS, FA], dt.float32, kind="ExternalInput").ap()
    c66 = nc.dram_tensor("c66", [N_CAT, FA], dt.float32, kind="ExternalInput").ap()
    wt = nc.dram_tensor("wt", [L, F, F], dt.float32, kind="ExternalInput").ap()
    bv = nc.dram_tensor("bv", [L, F], dt.float32, kind="ExternalInput").ap()
    # gather index columns: [3*T + NG_N, 128] — u, p, c, then neighbor cols
    idxp = nc.dram_tensor("idxp", [3 * T + NG_N, P], dt.int32, kind="ExternalInput").ap()
    invc = nc.dram_tensor("invc", [P, T], dt.float32, kind="ExternalInput").ap()
    nbad = nc.dram_tensor("nbad", [P, T], dt.float32, kind="ExternalInput").ap()
    hnot = nc.dram_tensor("hnot", [1, BC], dt.float32, kind="ExternalInput").ap()
    out_d = nc.dram_tensor("out", [T, P], dt.float32, kind="ExternalOutput").ap()

    f32 = dt.float32
    AX = mybir.AxisListType
    OP = mybir.AluOpType
    AF = mybir.ActivationFunctionType

    with tile.TileContext(nc) as tc, ExitStack() as ctx:
        sp = ctx.enter_context(tc.tile_pool(name="s", bufs=1))
        ixp = ctx.enter_context(tc.tile_pool(name="ix", bufs=48))
        ptp = ctx.enter_context(tc.tile_pool(name="ptp", bufs=2, space="PSUM"))
        pmm = ctx.enter_context(tc.tile_pool(name="pmm", bufs=2, space="PSUM"))

        ident = sp.tile([P, P], f32)
        make_identity(nc, ident[:])
        ones1 = sp.tile([1, F], f32)
        nc.vector.memset(ones1[:], 1.0)

        t_invc = sp.tile([P, T], f32)
        nc.sync.dma_start(out=t_invc[:], in_=invc[:, :])
        t_nbad = sp.tile([P, T], f32)
        nc.sync.dma_start(out=t_nbad[:], in_=nbad[:, :])
        t_hnot = sp.tile([1, BC], f32)
        nc.sync.dma_start(out=t_hnot[:], in_=hnot[:, :])
        t_wt = sp.tile([F, L, F], f32)
        nc.sync.dma_start(out=t_wt[:], in_=wt.rearrange("l i o -> i l o"))
        t_b = sp.tile([F, L], f32)
        nc.sync.dma_start(out=t_b[:], in_=bv.rearrange("l f -> f l"))

        # idx loads ride the Sync HWDGE queue only, keeping ACT free for the
        # dense phase to overlap the gather stream.
        def idx_tile(col):
            ix = ixp.tile([P, 1], dt.int32, name=f"ix{col}", tag="ix")
            nc.sync.dma_start(out=ix[:], in_=idxp[col, :].rearrange("(p o) -> p o", o=1))
            return ix

        def gather(out_tile, table, col, acc=False, bounds=None):
            ix = idx_tile(col)
            kw = {}
            if acc:
                kw["compute_op"] = OP.add
            if bounds is not None:
                kw["bounds_check"] = bounds
                kw["oob_is_err"] = False
            nc.gpsimd.indirect_dma_start(
                out=out_tile[:], out_offset=None, in_=table[:],
                in_offset=bass.IndirectOffsetOnAxis(ap=ix[:], axis=0), **kw)

        # ---- row gathers: u66 / p66(+c66) per slot --------------------
        t_u = [sp.tile([P, FA], f32, name=f"tu{t}", tag=f"tu{t}") for t in range(T)]
        t_pc = [sp.tile([P, FA], f32, name=f"tpc{t}", tag=f"tpc{t}") for t in range(T)]
        for t in range(T):
            gather(t_u[t], u66, t)
            gather(t_pc[t], p66, T + t)
            gather(t_pc[t], c66, 2 * T + t, acc=True)

        # row 0 of u66 broadcast to all partitions (for the clamp correction)
        t_u0row = sp.tile([1, F], f32)
        nc.sync.dma_start(out=t_u0row[:], in_=u66[0:1, 0:F])
        onesP = sp.tile([1, P], f32)
        nc.vector.memset(onesP[:], 1.0)
        pm0 = pmm.tile([P, F], f32, tag="mm0")
        nc.tensor.matmul(pm0[:], lhsT=onesP[:], rhs=t_u0row[:], start=True, stop=True)
        t_u0b = sp.tile([P, F], f32)
        nc.scalar.copy(out=t_u0b[:], in_=pm0[:])

        # ---- neighbor gathers (k-major interleave), sums on DVE -------
        # Plain gathers (no CCE-add: +650ns/instr, no bounds-check: +57ns);
        # padded slots are clamped to table row 0 on the host and their
        # aggregate contribution (nbad * u66[0]) is subtracted afterwards.
        t_acc = [sp.tile([P, F], f32, name=f"ta{t}", tag=f"ta{t}") for t in range(T)]
        gnp = ctx.enter_context(tc.tile_pool(name="gn", bufs=40))
        colbase = []
        c = 3 * T
        for t in range(T):
            colbase.append(c)
            c += k_sched[t]
        kmax = max(k_sched) if k_sched else 0
        for t in range(T):
            if k_sched[t] == 0:
                nc.vector.memset(t_acc[t][:], 0.0)
        ng = 0
        for k in range(kmax):
            for t in range(T):
                if k < k_sched[t]:
                    g = gnp.tile([P, FA], f32, name=f"g{ng}", tag="g")
                    ng += 1
                    gather(g, u66, colbase[t] + k)
                    if k == 0:
                        nc.vector.tensor_copy(out=t_acc[t][:], in_=g[:, 0:F])
                    else:
                        nc.vector.tensor_tensor(out=t_acc[t][:], in0=t_acc[t][:],
                                                in1=g[:, 0:F], op=OP.add)



        # ---- has_nbr==0 mask broadcast to [64, BC] via K=1 matmul -----
        # (copy_predicated wants an integer mask -> cast on the PSUM copy)
        t_m0 = sp.tile([F, BC], dt.uint8)
        for cch in range(NCH):
            pm = pmm.tile([F, CH], f32, tag="mm")
            nc.tensor.matmul(pm[:], lhsT=ones1[:], rhs=t_hnot[:, cch * CH:(cch + 1) * CH],
                             start=True, stop=True)
            nc.vector.tensor_copy(out=t_m0[:, cch * CH:(cch + 1) * CH], in_=pm[:])

        # ---- neighbor mean + transposes into feature-major ------------
        t_nm = sp.tile([P, T, F], f32)
        t_uT = sp.tile([F, BC], f32)
        t_nmT = sp.tile([F, BC], f32)
        for t in range(T):
            # nm = (acc - nbad*u0) * invc  ==  acc*invc - u0*(nbad*invc)
            tcor = gnp.tile([P, F], f32, name=f"cor{t}", tag="cor")
            nc.vector.tensor_scalar(
                out=tcor[:], in0=t_u0b[:],
                scalar1=t_nbad[:, t:t + 1], scalar2=t_invc[:, t:t + 1],
                op0=OP.mult, op1=OP.mult)
            nc.vector.tensor_scalar(
                out=t_nm[:, t, :], in0=t_acc[t][:],
                scalar1=t_invc[:, t:t + 1], scalar2=None, op0=OP.mult)
            nc.vector.tensor_tensor(
                out=t_nm[:, t, :], in0=t_nm[:, t, :], in1=tcor[:],
                op=OP.subtract)
            cs = slice(t * P, (t + 1) * P)
            pt = ptp.tile([F, P], f32, tag="tp")
            nc.tensor.transpose(out=pt[:], in_=t_u[t][:, 0:F], identity=ident[:])
            nc.scalar.copy(out=t_uT[:, cs], in_=pt[:])
            pt2 = ptp.tile([F, P], f32, tag="tp")
            nc.tensor.transpose(out=pt2[:], in_=t_nm[:, t, :], identity=ident[:])
            nc.scalar.copy(out=t_nmT[:, cs], in_=pt2[:])

        # ---- diffusion layers (feature-major) -------------------------
        t_xT = sp.tile([F, BC], f32)
        cur = t_uT
        nxt = sp.tile([F, BC], f32, name="t_uT2", tag="t_uT2")
        for l in range(L):
            for cch in range(NCH):
                cs = slice(cch * CH, (cch + 1) * CH)
                nc.vector.tensor_tensor(out=t_xT[:, cs], in0=cur[:, cs],
                                        in1=t_nmT[:, cs], op=OP.add)
                pm = pmm.tile([F, CH], f32, tag="mm")
                nc.tensor.matmul(pm[:], lhsT=t_wt[:, l, :], rhs=t_xT[:, cs],
                                 start=True, stop=True)
                nc.scalar.activation(out=nxt[:, cs], in_=pm[:], func=AF.Relu,
                                     bias=t_b[:, l:l + 1])
                nc.vector.copy_predicated(out=nxt[:, cs], mask=t_m0[:, cs],
                                          data=cur[:, cs])
            cur, nxt = nxt, cur

        # ---- back to row-major, 66-wide dot (biases included) ---------
        t_uf = sp.tile([P, T, FA], f32)
        for t in range(T):
            pt3 = ptp.tile([P, F], f32, tag="tb")
            nc.tensor.transpose(out=pt3[:], in_=cur[:, t * P:(t + 1) * P],
                                identity=ident[:F, :F])
            nc.scalar.copy(out=t_uf[:, t, 0:F], in_=pt3[:])
            nc.vector.tensor_copy(out=t_uf[:, t, F:FA], in_=t_u[t][:, F:FA])

        t_pcp = sp.tile([P, T, FA], f32)
        for t in range(T):
            nc.vector.tensor_copy(out=t_pcp[:, t, :], in_=t_pc[t][:])
        nc.vector.tensor_tensor(out=t_pcp[:], in0=t_pcp[:], in1=t_uf[:], op=OP.mult)
        t_int = sp.tile([P, T], f32)
        nc.vector.tensor_reduce(out=t_int[:], in_=t_pcp[:], axis=AX.X, op=OP.add)
        nc.sync.dma_start(out=out_d.rearrange("t p -> p t"), in_=t_int[:])

    nc.compile()
    return nc


_PROGRAM_CACHE = {}


def _get_program(k_sched):
    key = tuple(k_sched)
    if key not in _PROGRAM_CACHE:
        _PROGRAM_CACHE[key] = _build_program(key)
    return _PROGRAM_CACHE[key]


def kernel(user_idx, product_idx, category_idx, neighbor_idx, neighbor_lens,
           user_emb, product_emb, category_emb, user_bias_tab, product_bias_tab,
           global_bias, W, b, _run_kwargs=None, _return_res=False):
    user_idx = np.asarray(user_idx).astype(np.int32)
    product_idx = np.asarray(product_idx).astype(np.int32)
    category_idx = np.asarray(category_idx).astype(np.int32)
    neighbor_idx = np.asarray(neighbor_idx).astype(np.int32)
    neighbor_lens = np.asarray(neighbor_lens).astype(np.int64)
    user_emb = np.asarray(user_emb, dtype=np.float32)
    product_emb = np.asarray(product_emb, dtype=np.float32)
    category_emb = np.asarray(category_emb, dtype=np.float32)
    user_bias_tab = np.asarray(user_bias_tab, dtype=np.float32)
    product_bias_tab = np.asarray(product_bias_tab, dtype=np.float32)
    gb = float(np.asarray(global_bias, dtype=np.float32))
    W = np.asarray(W, dtype=np.float32)
    b = np.asarray(b, dtype=np.float32)

    # augmented tables: score = dot66(u66_final, p66+c66)
    u66_t = np.empty((N_USERS, FA), np.float32)
    u66_t[:, :F] = user_emb
    u66_t[:, F] = user_bias_tab + gb
    u66_t[:, F + 1] = 1.0
    p66_t = np.empty((N_PRODUCTS, FA), np.float32)
    p66_t[:, :F] = product_emb
    p66_t[:, F] = 1.0
    p66_t[:, F + 1] = product_bias_tab
    c66_t = np.zeros((N_CAT, FA), np.float32)
    c66_t[:, :F] = 0.3 * category_emb

    lens = np.clip(neighbor_lens, 0, K).astype(np.int64)

    # per-core sort by neighbor count; schedule shared across cores
    perms, kslots = [], np.zeros((NCORES, T), np.int64)
    for c in range(NCORES):
        lc = lens[c * BC:(c + 1) * BC]
        perm = np.argsort(lc, kind="stable")
        perms.append(perm)
        ls = lc[perm]
        kslots[c] = ls.reshape(T, P).max(axis=1)
    k_sched = tuple(int(x) for x in kslots.max(axis=0))
    nc = _get_program(k_sched)

    in_maps = []
    for c in range(NCORES):
        sl = slice(c * BC, (c + 1) * BC)
        perm = perms[c]
        ui = user_idx[sl][perm]
        pi = product_idx[sl][perm]
        ci = category_idx[sl][perm]
        ni = neighbor_idx[sl][perm]          # [BC, K]
        lc = lens[sl][perm]

        cols = [ui.reshape(T, P), pi.reshape(T, P), ci.reshape(T, P)]
        ncols = []
        ni3 = ni.reshape(T, P, K)
        lc2 = lc.reshape(T, P)
        for t in range(T):
            kk = k_sched[t]
            # padded slots clamp to table row 0; corrected via nbad on-chip
            col = np.where(np.arange(kk)[None, :] < lc2[t][:, None],
                           ni3[t, :, :kk], 0).astype(np.int32)  # [P, kk]
            ncols.append(col.T)              # [kk, P]
        idxp_np = np.concatenate([np.concatenate(cols, 0).astype(np.int32)]
                                 + ncols, axis=0)

        invc_np = (1.0 / np.maximum(lc2, 1)).astype(np.float32).T.copy()  # [P,T]
        nbad_np = (np.array(k_sched)[:, None] - lc2).astype(np.float32).T.copy()
        hnot_np = (lc == 0).astype(np.float32).reshape(1, BC)

        in_maps.append({
            "u66": u66_t, "p66": p66_t, "c66": c66_t,
            "wt": np.ascontiguousarray(W.transpose(0, 2, 1)),
            "bv": np.ascontiguousarray(b),
            "idxp": np.ascontiguousarray(idxp_np),
            "invc": np.ascontiguousarray(invc_np),
            "nbad": np.ascontiguousarray(nbad_np),
            "hnot": hnot_np,
        })

    res = run_bass_kernel_spmd(nc, in_maps, list(range(NCORES)),
                               **(_run_kwargs or {}))
    out = np.empty(B, np.float32)
    for c in range(NCORES):
        o = res.results[c]["out"].reshape(-1)   # sorted order, r = t*128+p
        dst = out[c * BC:(c + 1) * BC]
        dst[perms[c]] = o
    if _return_res:
        return out, res
    return out
